# revision 24
# baseline (speedup 1.0000x reference)
"""Multi-head self-attention (B=4, S=2048, D=1024, H=16, Hd=64) on 8 TRN2 cores.

Sharding: core c -> (batch b = c//2, head-group g = c%2 of 8 heads).
Each core computes its batch's 8 heads end-to-end plus the partial output
projection for its head group; the host sums the two head-group partials
per batch. No collectives.

Device layout is fully transposed: activations are [feature(partitions),
seq(free)], so the matmul chain QKV -> scores -> PV -> out-proj needs no
on-device transposes. The softmax k-sum comes from an all-ones column
appended to each head's V slice (denominator lands in PSUM partition 64
of the PV output).

Schedule: the ACT engine's exp over all scores (~270us) is the hard
bottleneck, so the program is emitted in an order that keeps it saturated:
QK projection for head-pair 0 first, then attention blocks; V projection,
the remaining QK pairs and the per-qb output projection are emitted as
lower-priority PE filler. Scores matmuls (K=64) are packed two heads at a
time onto PE row-halves via tile_position so both heads' score tiles
stream concurrently.
"""

from contextlib import ExitStack

import numpy as np
import ml_dtypes

import concourse.bass as bass
import concourse.tile as tile
from concourse import mybir
from concourse.bass_utils import run_bass_kernel_spmd
from concourse.vector_clock import ScopedClock
from bass_rust import InstNoOp, SyncInfo

BF16 = mybir.dt.bfloat16
F32 = mybir.dt.float32
AF = mybir.ActivationFunctionType

B, S, D = 4, 2048, 1024
H, HD = 16, 64
GH = 8          # heads per core (head-group size)
GM = GH * HD    # 512 head dims per core
NQB = 4         # q blocks of 512
QB = 512
NKC = 16        # k chunks of 128
NDC = 8         # d chunks of 128 (contraction for projections)
VW = GH * (HD + 1)  # 520: per-k-chunk V slice (8 heads x (64 dims + ones col))

_META_TYPES = ("TileBranchInst", "BassTileLoopBlock", "BassTilePoolBoundary")


class _TileCtx(tile.TileContext):
    """Splits multi-sem-wait instructions: the pinned walrus rejects any TPB
    instruction carrying more than one sem-wait, while Tile emits joins and a
    global end-of-context drain with several."""

    def _split_waits(self, ordered):
        nc = self.nc
        for bb_name, insts in ordered.items():
            out = []
            for inst in insts:
                si = inst.sync_info
                if (
                    si is not None
                    and si.on_wait
                    and len(si.on_wait) > 1
                    and type(inst).__name__ not in _META_TYPES
                    and inst.engine != mybir.EngineType.Unassigned
                ):
                    waits = list(si.on_wait)
                    for w in waits[:-1]:
                        nop = InstNoOp(
                            name=nc.get_next_instruction_name(), ins=[], outs=[]
                        )
                        nop.engine = inst.engine
                        nop.sync_info = SyncInfo(on_wait=[w], on_update=[])
                        out.append(nop)
                    inst.sync_info = SyncInfo(
                        on_wait=[waits[-1]], on_update=list(si.on_update)
                    )
                out.append(inst)
            ordered[bb_name] = out

    def _lower_ordered_insts(self, ordered):
        self._split_waits(ordered)
        super()._lower_ordered_insts(ordered)

    def _drain_and_barrier(self, tick_clock, wait_clock):
        drain_inst = self.nc.sync.drain()
        wait_clock.add_sem_waits(
            drain_inst.ins, ScopedClock({None: tick_clock.global_clock})
        )
        si = drain_inst.ins.sync_info
        waits = list(si.on_wait) if si is not None else []
        if len(waits) > 1:
            drain_inst.ins.sync_info = SyncInfo(
                on_wait=waits[:1], on_update=list(si.on_update)
            )
            for w in waits[1:]:
                extra = self.nc.sync.drain()
                extra.ins.sync_info = SyncInfo(on_wait=[w], on_update=[])

        self.nc.all_engine_barrier()
        assert self.sems is not None
        popped = self.nc._tile_sem_poison_stack.pop()
        assert popped is self._sem_poison
        self.nc.clear_and_free_semaphores(list(self.sems.allocated().values()))
        self.nc.all_engine_barrier()


def _build_program():
    nc = bass.Bass(trn_type="TRN2", debug=False, num_devices=8)

    xT = nc.dram_tensor("xT", [D, S], BF16, kind="ExternalInput").ap()
    wq = nc.dram_tensor("wq", [D, GM], BF16, kind="ExternalInput").ap()
    wk = nc.dram_tensor("wk", [D, GM], BF16, kind="ExternalInput").ap()
    wv = nc.dram_tensor("wv", [D, GM], BF16, kind="ExternalInput").ap()
    # pair-major-reordered Wo.T slice: [128, 4 pairs x 1024]
    wo = nc.dram_tensor("wo", [128, (GM // 128) * D], BF16, kind="ExternalInput").ap()
    bq = nc.dram_tensor("bq", [GM], F32, kind="ExternalInput").ap()
    bk = nc.dram_tensor("bk", [GM], F32, kind="ExternalInput").ap()
    bo = nc.dram_tensor("bo", [D], F32, kind="ExternalInput").ap()
    outT = nc.dram_tensor("outT", [D, S], F32, kind="ExternalOutput").ap()

    with _TileCtx(nc) as tc, ExitStack() as ctx:
        const_pool = ctx.enter_context(tc.tile_pool(name="const", bufs=1))
        act_pool = ctx.enter_context(tc.tile_pool(name="acts", bufs=1))

        # ---- persistent SBUF tiles -------------------------------------
        bq_sb = const_pool.tile([128, GM // 128], F32, tag="bq")
        bk_sb = const_pool.tile([128, GM // 128], F32, tag="bk")
        bo_sb = const_pool.tile([128, NDC], F32, tag="bo")
        wo_sb = const_pool.tile([128, (GM // 128) * D], BF16, tag="wo")
        xt = const_pool.tile([128, NDC * S], BF16, tag="xt")
        wq_sb = const_pool.tile([128, NDC * GM], BF16, tag="wq")
        wk_sb = const_pool.tile([128, NDC * GM], BF16, tag="wk")
        wv_sb = const_pool.tile([128, NDC * GM], BF16, tag="wv")
        qt = act_pool.tile([128, (GM // 128) * S], BF16, tag="qt")
        kt = act_pool.tile([128, (GM // 128) * S], BF16, tag="kt")
        v_sb = act_pool.tile([128, NKC * VW], BF16, tag="v")
        otp = [
            act_pool.tile([128, S], BF16, name=f"otp{t}", tag=f"otp{t}")
            for t in range(GH // 2)
        ]

        # ---- input DMAs (emission order = scheduler priority) ----------
        def _w_pair_dma(dst, src, mi):
            nc.sync.dma_start(
                dst[:].rearrange("p (c m) -> p c m", m=GM)[
                    :, :, mi * 128 : (mi + 1) * 128
                ],
                src.rearrange("(c p) m -> p c m", p=128)[
                    :, :, mi * 128 : (mi + 1) * 128
                ],
            )

        _w_pair_dma(wk_sb, wk, 0)
        # x pieces qb-major so the first projection groups unblock early;
        # qb0 entirely on the sync HWDGE ring (gpsimd SWDGE pays a ~6us
        # first-use IRAM load), later pieces alternate queues.
        for dc in range(NDC):
            nc.sync.dma_start(
                xt[:, dc * S : dc * S + QB],
                xT[dc * 128 : (dc + 1) * 128, 0:QB],
            )
        _w_pair_dma(wq_sb, wq, 0)
        nc.gpsimd.dma_start(
            wv_sb[:].rearrange("p (c m) -> p c m", m=GM),
            wv.rearrange("(c p) m -> p c m", p=128),
        )
        nc.gpsimd.dma_start(bq_sb[:], bq.rearrange("(c p) -> p c", p=128))
        nc.gpsimd.dma_start(bk_sb[:], bk.rearrange("(c p) -> p c", p=128))
        nc.gpsimd.dma_start(bo_sb[:], bo.rearrange("(c p) -> p c", p=128))
        for qb in range(1, NQB):
            for dc in range(NDC):
                eng = nc.sync if (dc % 2 == 0) else nc.gpsimd
                eng.dma_start(
                    xt[:, dc * S + qb * QB : dc * S + (qb + 1) * QB],
                    xT[dc * 128 : (dc + 1) * 128, qb * QB : (qb + 1) * QB],
                )
        for mi in range(1, GM // 128):
            _w_pair_dma(wk_sb, wk, mi)
            _w_pair_dma(wq_sb, wq, mi)
        nc.gpsimd.dma_start(wo_sb[:], wo[:, :])

        # ---- working pools ---------------------------------------------
        s_psum = ctx.enter_context(
            tc.tile_pool(name="s_psum", bufs=2, space="PSUM")
        )  # [128,1024] scores tiles (2 banks each)
        sp_small = ctx.enter_context(
            tc.tile_pool(name="sp_small", bufs=4, space="PSUM")
        )  # [128,512] one-bank tiles: projections, PV out, out-proj
        slab_pool = ctx.enter_context(tc.tile_pool(name="slab", bufs=24))
        pos_pool = ctx.enter_context(tc.tile_pool(name="pos", bufs=3))
        den_pool = ctx.enter_context(tc.tile_pool(name="den", bufs=3))
        bc_pool = ctx.enter_context(tc.tile_pool(name="bc", bufs=3))
        tmp_pool = ctx.enter_context(tc.tile_pool(name="tmp", bufs=2))
        y_pool = ctx.enter_context(tc.tile_pool(name="y", bufs=2))
        dram_pool = ctx.enter_context(
            tc.tile_pool(name="dscr", bufs=6, space="DRAM")
        )

        def emit_qk_group(mi, which, qb):
            """Project one head pair's Q or K for one q block: 8 matmuls."""
            w_sb, b_sb, dst = (
                (wq_sb, bq_sb, qt) if which == "q" else (wk_sb, bk_sb, kt)
            )
            ps = sp_small.tile([128, QB], F32, tag="sm")
            lhs_off = mi * 128
            for dc in range(NDC):
                nc.tensor.matmul(
                    ps[:],
                    w_sb[:, dc * GM + lhs_off : dc * GM + lhs_off + 128],
                    xt[:, dc * S + qb * QB : dc * S + (qb + 1) * QB],
                    start=(dc == 0),
                    stop=(dc == NDC - 1),
                )
            nc.vector.tensor_scalar_add(
                dst[:, mi * S + qb * QB : mi * S + (qb + 1) * QB],
                ps[:],
                b_sb[:, mi : mi + 1],
            )

        def emit_qk_pair(mi):
            for qb in range(NQB):
                emit_qk_group(mi, "k", qb)
                emit_qk_group(mi, "q", qb)

        def emit_v_chunk(si):
            ps = sp_small.tile([128, GM], F32, tag="sm")
            for dc in range(NDC):
                nc.tensor.matmul(
                    ps[:],
                    xt[:, dc * S + si * 128 : dc * S + (si + 1) * 128],
                    wv_sb[:, dc * GM : (dc + 1) * GM],
                    start=(dc == 0),
                    stop=(dc == NDC - 1),
                )
            nc.vector.tensor_copy(
                v_sb[:, si * VW : (si + 1) * VW]
                .rearrange("p (h m) -> p h m", h=GH)[:, :, 0:HD],
                ps[:].rearrange("p (h m) -> p h m", h=GH),
            )

        def emit_scores(t, qb):
            """Scores + exp for head pair (2t, 2t+1), q block qb -> slab list."""
            qsl0 = slice(t * S + qb * QB, t * S + (qb + 1) * QB)
            slabs = []
            for kc in range(NKC):
                ksl = slice(t * S + kc * 128, t * S + (kc + 1) * 128)
                sp = s_psum.tile([128, 2 * QB], F32, tag="sp")
                nc.tensor.matmul(
                    sp[:, 0:QB],
                    kt[0:64, ksl],
                    qt[0:64, qsl0],
                    start=True,
                    stop=True,
                    tile_position=(0, 0),
                )
                nc.tensor.matmul(
                    sp[:, QB : 2 * QB],
                    kt[64:128, ksl],
                    qt[64:128, qsl0],
                    start=True,
                    stop=True,
                    tile_position=(64, 0),
                )
                sl = slab_pool.tile([128, 2 * QB], BF16, tag="slab")
                nc.scalar.activation(sl[:], sp[:], AF.Exp, scale=0.125)
                slabs.append(sl)
            return slabs

        def emit_pv_mms(t, qb, slabs, po, kc_range):
            """PV accumulation matmuls; heads interleave per k-chunk so each
            slab is released right after its pair of matmuls."""
            for kc in kc_range:
                for i in range(2):
                    h = 2 * t + i
                    nc.tensor.matmul(
                        po[i][0 : HD + 1, :],
                        v_sb[
                            :,
                            kc * VW + h * (HD + 1) : kc * VW + (h + 1) * (HD + 1),
                        ],
                        slabs[kc][:, i * QB : (i + 1) * QB],
                        start=(kc == 0),
                        stop=(kc == NKC - 1),
                    )

        def emit_pv_norm(t, qb, po):
            # norm-chain DMAs ride the idle gpsimd SWDGE queue (sync carries
            # the streaming input/output traffic)
            dq = nc.gpsimd
            for i in range(2):
                # evacuate PSUM promptly, then normalize off the critical path
                po_s = pos_pool.tile([128, QB], F32, tag="pos")
                nc.vector.tensor_copy(po_s[0 : HD + 1, :], po[i][0 : HD + 1, :])
                # DVE reciprocal is free-dim-serial per lane (~6.4 ns/elem),
                # so spread the 512 denominators over all 128 partitions via
                # a DRAM bounce: [1,512] -> [128,4] -> recip -> broadcast.
                scr = dram_pool.tile([QB], F32, tag="scr")
                dq.dma_start(scr.unsqueeze(0), po_s[HD : HD + 1, :])
                dsq = den_pool.tile([128, QB // 128], F32, tag="dsq")
                dq.dma_start(dsq[:], scr.rearrange("(p f) -> p f", p=128))
                drq = den_pool.tile([128, QB // 128], F32, tag="drq")
                nc.vector.reciprocal(drq[:], dsq[:])
                scr2 = dram_pool.tile([QB], F32, tag="scr2")
                dq.dma_start(scr2.rearrange("(p f) -> p f", p=128), drq[:])
                bcast = bc_pool.tile([HD, QB], F32, tag="bcast")
                dq.dma_start(
                    bcast[:], scr2.unsqueeze(0).broadcast_to([HD, QB])
                )
                if i == 0:
                    nc.vector.tensor_mul(
                        otp[t][0:HD, qb * QB : (qb + 1) * QB],
                        po_s[0:HD, :],
                        bcast[:],
                    )
                else:
                    tmp = tmp_pool.tile([HD, QB], BF16, tag="tmp")
                    nc.vector.tensor_mul(tmp[:], po_s[0:HD, :], bcast[:])
                    dq.dma_start(
                        otp[t][HD:128, qb * QB : (qb + 1) * QB], tmp[:]
                    )

        def emit_pv(t, qb, slabs, kc_range=None, po=None):
            if po is None:
                po = [
                    sp_small.tile([128, QB], F32, name=f"po{i}", tag="sm")
                    for i in range(2)
                ]
            emit_pv_mms(t, qb, slabs, po, kc_range or range(NKC))
            return po

        def emit_outproj(qb):
            for ec in range(NDC):
                ps = sp_small.tile([128, QB], F32, tag="sm")
                for mt in range(GM // 128):
                    nc.tensor.matmul(
                        ps[:],
                        wo_sb[:, mt * D + ec * 128 : mt * D + (ec + 1) * 128],
                        otp[mt][:, qb * QB : (qb + 1) * QB],
                        start=(mt == 0),
                        stop=(mt == GM // 128 - 1),
                    )
                y_sb = y_pool.tile([128, QB], F32, tag="y")
                nc.vector.tensor_scalar_add(y_sb[:], ps[:], bo_sb[:, ec : ec + 1])
                nc.sync.dma_start(
                    outT[ec * 128 : (ec + 1) * 128, qb * QB : (qb + 1) * QB],
                    y_sb[:],
                )

        # ---- emission schedule -----------------------------------------
        # Emission order doubles as (a) the sequential-semantics dependency
        # order and (b) the scheduler priority.  The attention blocks are
        # software-pipelined one deep: block i+1's scores (which feed the
        # bottleneck ACT engine) are emitted BEFORE block i's PV, so exps
        # never starve.  V projection and the remaining QK pairs slot in as
        # lower-priority PE filler; out-proj runs per-q-block as soon as the
        # last head pair lands.
        nc.vector.memset(v_sb[:], 1.0)  # ones columns for the denominator
        emit_qk_pair(0)
        blocks = [(t, qb) for t in range(GH // 2) for qb in range(NQB)]
        slabs = {b: None for b in blocks}
        slabs[(0, 0)] = emit_scores(0, 0)
        slabs[(0, 1)] = emit_scores(0, 1)
        # V projection here: the first two blocks have no PV work yet, so
        # their exp-wait gaps (~12us each) absorb these 128 matmuls while
        # the first exps stream.
        for si in range(NKC):
            emit_v_chunk(si)
        for i, b in enumerate(blocks):
            t, qb = b
            if i + 2 < len(blocks):
                slabs[blocks[i + 2]] = emit_scores(*blocks[i + 2])
            po = emit_pv(t, qb, slabs[b])
            emit_pv_norm(t, qb, po)
            if qb == 1 and t < GH // 2 - 1:
                # pair t+1's projection as filler — must precede the
                # emission of scores(t+1, 0) two slots ahead
                emit_qk_pair(t + 1)
        # out-proj last: lowest priority, so it only gap-fills; each qb's
        # groups become ready as soon as pair 3's norms for that qb land.
        for qb in range(NQB):
            emit_outproj(qb)

    return nc


_NC = None
_last_in_maps = None


def _get_program():
    global _NC
    if _NC is None:
        _NC = _build_program()
    return _NC


def kernel(x, Wq, bq, Wk, bk, Wv, bv, Wo, bo):
    x = np.asarray(x, np.float32)
    bf = ml_dtypes.bfloat16
    in_maps = []
    for c in range(8):
        b, g = c // 2, c % 2
        sl = slice(g * GM, (g + 1) * GM)
        wo_slice = np.asarray(Wo, np.float32)[:, sl].T  # [512, 1024]
        # fold bv and half of bo into the output bias
        bo_eff = np.asarray(bo, np.float32) / 2.0 + np.asarray(bv, np.float32)[sl] @ wo_slice
        in_maps.append(
            {
                "xT": np.ascontiguousarray(x[b].T).astype(bf),
                "wq": np.ascontiguousarray(np.asarray(Wq, np.float32)[sl, :].T).astype(bf),
                "wk": np.ascontiguousarray(np.asarray(Wk, np.float32)[sl, :].T).astype(bf),
                "wv": np.ascontiguousarray(np.asarray(Wv, np.float32)[sl, :].T).astype(bf),
                "wo": np.ascontiguousarray(
                    wo_slice.reshape(GM // 128, 128, D).transpose(1, 0, 2).reshape(128, (GM // 128) * D)
                ).astype(bf),
                "bq": np.ascontiguousarray(np.asarray(bq, np.float32)[sl]),
                "bk": np.ascontiguousarray(np.asarray(bk, np.float32)[sl]),
                "bo": np.ascontiguousarray(bo_eff.astype(np.float32)),
            }
        )

    global _last_in_maps
    _last_in_maps = in_maps
    nc = _get_program()
    res = run_bass_kernel_spmd(nc, in_maps, core_ids=list(range(8)))
    out = np.empty((B, S, D), np.float32)
    for b in range(B):
        acc = res.results[2 * b]["outT"].astype(np.float32) + res.results[
            2 * b + 1
        ]["outT"].astype(np.float32)
        out[b] = acc.T
    return out
